# revision 6
# baseline (speedup 1.0000x reference)
"""AttentionGNN (A3TGCN) Trainium2 kernel — self-contained.

Math restructuring (exact):
  GCNConv is linear and A_hat = D^-1/2 (A+I) D^-1/2 is fixed across the 12
  timesteps and 3 gates, so the sparse aggregation is done ONCE on the
  stacked features X' = dinv * x  (shape [N, 192], t-major columns):
      Y = dinv * ((A+I) @ X')
  Everything downstream (GRU cell per timestep, attention sum, head) is
  dense [N,32] work.

Distribution: nodes are partitioned into 8 contiguous dst-ranges (one per
NeuronCore).  Every core receives the full prescaled X' in HBM as fp8
(e4m3, 256B rows) and gathers the rows for its ~400k incoming edges with
dma_gather, reducing them per 127-node dst windows via one-hot matmuls
(fp8) into PSUM.  One-hot scatter matrices are built 16 chunks at a time
with a single broadcast-AP is_equal on the vector engine.  The GRU time
loop runs on the transposed [feat, node] layout, split into 4
node-contiguous quarters so it pipelines into the gather shadow.

Host-side work is limited to integer planning (bucketing / padding edge
lists) and the D^-1/2 prescale; all O(E*F) float work runs on the
NeuronCores.
"""

import math
from contextlib import ExitStack

import numpy as np

try:
    import ml_dtypes

    BF16 = ml_dtypes.bfloat16
    F8NP = ml_dtypes.float8_e4m3fn
except ImportError:  # pragma: no cover
    BF16 = None
    F8NP = None

N_GLOBAL = 100_000
F_IN = 16
T = 12
HID = 32
N_CLS = 2
NCORES = 8


class Cfg:
    def __init__(self, n, b_windows=4, gsz=400, nq=8):
        assert n % NCORES == 0
        self.N = n
        self.NLOC = n // NCORES
        self.W = 127                      # real dst nodes per window
        self.NW = math.ceil(self.NLOC / self.W)
        self.B = b_windows                # windows per PSUM batch
        self.NB = math.ceil(self.NW / self.B)
        self.KCH = 4                      # src chunks (int16 gather indices)
        self.CHROWS = math.ceil(n / self.KCH)
        assert self.CHROWS <= 32767
        self.GSZ = gsz                    # GRU group size (nodes)
        self.NQ = nq                      # GRU quads (groups = 4*NQ)
        self.NODE_PAD = 4 * nq * gsz
        assert self.NODE_PAD >= self.W * self.NW + 1
        self.FP = 192                     # real feature columns (t*16+f)
        self.FPAD = 256                   # padded row length (256B fp8)


FULL = Cfg(N_GLOBAL, b_windows=4, gsz=400, nq=8)


# ---------------------------------------------------------------- planning

def plan(cfg, edge_index):
    """Bucket edges (plus self loops) per core into (window g, src-chunk k)
    buckets, pad each bucket to a multiple of 128 slots that is uniform
    across cores, and emit flat idx16 / dstl arrays in processing order:
       for batch b: for k: for window g in batch: bucket slots.
    Returns (shared structure, per-core arrays)."""
    src = np.concatenate([edge_index[0], np.arange(cfg.N, dtype=np.int64)])
    dst = np.concatenate([edge_index[1], np.arange(cfg.N, dtype=np.int64)])
    src = src.astype(np.int64)
    dst = dst.astype(np.int64)

    core = dst // cfg.NLOC
    NWK = cfg.NW * cfg.KCH
    counts = np.zeros((NCORES, NWK), dtype=np.int64)
    per_core = []
    for c in range(NCORES):
        m = core == c
        s = src[m]
        d = dst[m] - c * cfg.NLOC
        g = d // cfg.W
        dstl = (d - g * cfg.W).astype(np.int64)
        k = s // cfg.CHROWS
        i16 = (s - k * cfg.CHROWS).astype(np.int64)
        bidx = g // cfg.B
        order = np.lexsort((g, k, bidx))
        g, dstl, k, i16 = g[order], dstl[order], k[order], i16[order]
        bucket = g * cfg.KCH + k
        counts[c] = np.bincount(bucket, minlength=NWK)
        per_core.append((bucket, dstl, i16))

    maxcnt = counts.max(axis=0)
    nchunks = -(-maxcnt // 128)           # ceil, per (g,k) flattened g*KCH+k
    assert (nchunks > 0).all()
    slots = nchunks * 128

    # bucket processing order and offsets
    order_buckets = []                    # flattened (g*KCH+k) in emit order
    for b in range(cfg.NB):
        gs = range(b * cfg.B, min((b + 1) * cfg.B, cfg.NW))
        for k in range(cfg.KCH):
            for g in gs:
                order_buckets.append(g * cfg.KCH + k)
    order_buckets = np.array(order_buckets, dtype=np.int64)
    off_in_order = np.zeros(NWK, dtype=np.int64)
    off_in_order[order_buckets] = np.concatenate(
        [[0], np.cumsum(slots[order_buckets])[:-1]]
    )
    tot = int(slots.sum())

    idx16_all = np.zeros((NCORES, tot), dtype=np.int16)
    dstl_all = np.full((NCORES, tot), cfg.W, dtype=np.int16)  # pad -> dead row
    for c in range(NCORES):
        bucket, dstl, i16 = per_core[c]
        # rank within bucket (buckets appear as contiguous runs after sort,
        # but NOT in bucket-id order — use run change points)
        chg = np.flatnonzero(np.r_[True, bucket[1:] != bucket[:-1]])
        run_start_pos = np.repeat(chg, np.diff(np.r_[chg, bucket.size]))
        rank = np.arange(bucket.size, dtype=np.int64) - run_start_pos
        pos = off_in_order[bucket] + rank
        idx16_all[c, pos] = i16.astype(np.int16)
        dstl_all[c, pos] = dstl.astype(np.int16)

    shared = {
        "nchunks": nchunks,               # [NW*KCH]
        "tot": tot,
        "order_buckets": order_buckets,
        "off": off_in_order,
    }
    return shared, idx16_all, dstl_all


# ---------------------------------------------------------------- builder

def build_program(cfg, shared, probs, head_bd, phases=3):
    """Build the SPMD Bacc program (identical for all cores)."""
    from concourse import bacc, mybir
    from concourse.bass import AP
    import concourse.tile as tile

    bf = mybir.dt.bfloat16
    f8 = mybir.dt.float8e4
    f32 = mybir.dt.float32
    i16 = mybir.dt.int16
    AF = mybir.ActivationFunctionType
    ALU = mybir.AluOpType

    nchunks = shared["nchunks"]
    tot = shared["tot"]

    nc = bacc.Bacc("TRN2", target_bir_lowering=False, debug=False,
                   enable_asserts=False, num_devices=NCORES)

    # ---- DRAM I/O
    xp = nc.dram_tensor("xp", [cfg.N, cfg.FPAD], f8, kind="ExternalInput")
    idxw = nc.dram_tensor("idxw", [128, tot // 16], i16, kind="ExternalInput")
    dstl = nc.dram_tensor("dstl", [128, tot // 128], bf, kind="ExternalInput")
    dinv = nc.dram_tensor("dinv", [128, cfg.NW], f32, kind="ExternalInput")
    w1 = nc.dram_tensor("w1big", [128, 1152], bf, kind="ExternalInput")
    w2 = nc.dram_tensor("w2big", [128, 384], bf, kind="ExternalInput")
    bzd = nc.dram_tensor("bz", [128, 1], f32, kind="ExternalInput")
    brd = nc.dram_tensor("br", [128, 1], f32, kind="ExternalInput")
    bhd = nc.dram_tensor("bh", [128, 1], f32, kind="ExternalInput")
    hdd = nc.dram_tensor("hd", [128, 4], bf, kind="ExternalInput")
    iod = nc.dram_tensor("iota", [128, 16 * 128], bf, kind="ExternalInput")
    idd = nc.dram_tensor("ident", [128, 128], bf, kind="ExternalInput")
    out = nc.dram_tensor("out", [1, cfg.NODE_PAD], bf, kind="ExternalOutput")
    dbg = (nc.dram_tensor("dbg", [128, cfg.NODE_PAD], bf, kind="ExternalOutput")
           if phases < 3 else None)

    NP = cfg.NODE_PAD
    GSZ, NQ = cfg.GSZ, cfg.NQ

    # ---- persistent SBUF tensors
    A_t = nc.alloc_sbuf_tensor("A_t", [128, NP], bf).ap()     # t0..t7 feats
    B_t = nc.alloc_sbuf_tensor("B_t", [64, NP], bf).ap()      # t8..t11 feats
    H = nc.alloc_sbuf_tensor("H", [128, NQ * GSZ], bf).ap()
    Z = nc.alloc_sbuf_tensor("Z", [128, NQ * GSZ], bf).ap()
    HC = nc.alloc_sbuf_tensor("HC", [128, NQ * GSZ], bf).ap()
    ACC = nc.alloc_sbuf_tensor("ACCt", [128, NQ * GSZ], bf).ap()
    T1 = nc.alloc_sbuf_tensor("T1", [128, NQ * GSZ], bf).ap()
    T2 = nc.alloc_sbuf_tensor("T2", [128, NQ * GSZ], bf).ap()
    W1 = nc.alloc_sbuf_tensor("W1", [128, 1152], bf).ap()
    W2 = nc.alloc_sbuf_tensor("W2", [128, 384], bf).ap()
    BZ = nc.alloc_sbuf_tensor("BZ", [128, 1], f32).ap()
    BR = nc.alloc_sbuf_tensor("BR", [128, 1], f32).ap()
    BH = nc.alloc_sbuf_tensor("BH", [128, 1], f32).ap()
    HD = nc.alloc_sbuf_tensor("HD", [128, 4], bf).ap()
    IOTA = nc.alloc_sbuf_tensor("IOTA", [128, 16, 128], bf).ap()
    IDN = nc.alloc_sbuf_tensor("IDN", [128, 128], bf).ap()
    DINV = nc.alloc_sbuf_tensor("DINV", [128, cfg.NW], f32).ap()
    DSTL = nc.alloc_sbuf_tensor("DSTL", [128, tot // 128], bf).ap()

    # gather tile sizing: max chunks for one (batch, k) gather
    bk_chunks = []                        # [(b, k, nidx, [(g, nch), ...])]
    coff = 0
    ioff = 0
    for b in range(cfg.NB):
        gs = list(range(b * cfg.B, min((b + 1) * cfg.B, cfg.NW)))
        for k in range(cfg.KCH):
            wins = [(g, int(nchunks[g * cfg.KCH + k])) for g in gs]
            nch = sum(w[1] for w in wins)
            bk_chunks.append((b, k, nch, wins, coff, ioff))
            coff += nch
            ioff += nch * 128
    maxnc = max(e[2] for e in bk_chunks)

    # batch -> (chunk col range) for one-hot building
    batch_cols = {}
    for (b, k, nch, wins, coff_, ioff_) in bk_chunks:
        lo, hi = batch_cols.get(b, (coff_, coff_ + nch))
        batch_cols[b] = (min(lo, coff_), max(hi, coff_ + nch))

    with tile.TileContext(nc) as tc:
        with tc.tile_pool(name="cpool", bufs=1) as cpool:
            # ---- load constants
            nc.sync.dma_start(W1, w1.ap())
            nc.sync.dma_start(W2, w2.ap())
            nc.sync.dma_start(BZ, bzd.ap())
            nc.sync.dma_start(BR, brd.ap())
            nc.sync.dma_start(BH, bhd.ap())
            nc.sync.dma_start(HD, hdd.ap())
            nc.sync.dma_start(IOTA[:, :, :],
                              AP(iod, 0, [[iod.ap().ap[0][0], 128],
                                          [128, 16], [1, 128]]))
            nc.sync.dma_start(IDN, idd.ap())
            nc.sync.dma_start(DINV, dinv.ap())
            nc.sync.dma_start(DSTL, dstl.ap())
            nc.gpsimd.memset(H, 0)
            nc.gpsimd.memset(ACC, 0)
            tail = cfg.W * cfg.NW + 1
            if tail < NP:
                nc.gpsimd.memset(A_t[:, tail - 128:NP], 0)
                nc.gpsimd.memset(B_t[:, tail - 128:NP], 0)

            # ================= phase 1: aggregation =================
            xap = xp.ap()
            p1 = ExitStack()
            gpool = p1.enter_context(tc.tile_pool(name="gpool", bufs=3))
            ipool = p1.enter_context(tc.tile_pool(name="ipool", bufs=3))
            spool = p1.enter_context(tc.tile_pool(name="spool", bufs=4))
            ypool = p1.enter_context(tc.tile_pool(name="ypool", bufs=3))
            wpsp = p1.enter_context(
                tc.tile_pool(name="wps", bufs=4, space="PSUM"))
            tpsp = p1.enter_context(
                tc.tile_pool(name="tps", bufs=1, space="PSUM"))

            # phase-2 pools (allocated up front; used interleaved per quarter)
            rpool = p1.enter_context(tc.tile_pool(name="rpool", bufs=2))
            opool = p1.enter_context(tc.tile_pool(name="opool", bufs=2))
            zrpsp = p1.enter_context(
                tc.tile_pool(name="zrps", bufs=1, space="PSUM"))
            hcpsp = p1.enter_context(
                tc.tile_pool(name="hcps", bufs=1, space="PSUM"))

            def emit_phase2_quarter(Q):
                """GRU + attention + head for quads 2Q, 2Q+1 (nodes
                [Q*2*4*GSZ, (Q+1)*2*4*GSZ) in block layout)."""
                if phases < 2:
                    return
                qcols = slice(2 * Q * GSZ, (2 * Q + 2) * GSZ)
                for t in range(T):
                    Ysrc = A_t if t < 8 else B_t
                    KH = 128 if t < 8 else 64
                    tp0 = 0
                    wz = W1[0:KH, (t * 3 + 0) * 32:(t * 3 + 1) * 32]
                    wr = W1[0:KH, (t * 3 + 1) * 32:(t * 3 + 2) * 32]
                    wh = W1[0:KH, (t * 3 + 2) * 32:(t * 3 + 3) * 32]
                    for q in (2 * Q, 2 * Q + 1):
                        qc = slice(q * GSZ, (q + 1) * GSZ)
                        zt = zrpsp.tile([128, 512], f32, space="PSUM",
                                        tag="zt")
                        rt = zrpsp.tile([128, 512], f32, space="PSUM",
                                        tag="rt")
                        hcp = hcpsp.tile([128, 512], f32, space="PSUM",
                                         tag="hc")
                        for s in range(4):
                            # block layout: quad q, band s -> node block
                            # 8*(q//2) + (q%2) + 2*s
                            blk = 8 * (q // 2) + (q % 2) + 2 * s
                            yv = Ysrc[0:KH, blk * GSZ:(blk + 1) * GSZ]
                            r0 = slice(32 * s, 32 * s + 32)
                            ws = slice((s * 3) * 32, (s * 3 + 1) * 32)
                            nc.tensor.matmul(zt[r0, 0:GSZ], lhsT=wz, rhs=yv,
                                             start=True, stop=False,
                                             skip_group_check=True,
                                             tile_position=(tp0, 32 * s))
                            nc.tensor.matmul(zt[r0, 0:GSZ],
                                             lhsT=W2[:, ws],
                                             rhs=H[:, qc],
                                             start=False, stop=True,
                                             skip_group_check=True,
                                             tile_position=(0, 32 * s))
                            ws = slice((s * 3 + 1) * 32, (s * 3 + 2) * 32)
                            nc.tensor.matmul(rt[r0, 0:GSZ], lhsT=wr,
                                             rhs=yv, start=True, stop=False,
                                             skip_group_check=True,
                                             tile_position=(tp0, 32 * s))
                            nc.tensor.matmul(rt[r0, 0:GSZ],
                                             lhsT=W2[:, ws],
                                             rhs=H[:, qc],
                                             start=False, stop=True,
                                             skip_group_check=True,
                                             tile_position=(0, 32 * s))
                            nc.tensor.matmul(hcp[r0, 0:GSZ], lhsT=wh, rhs=yv,
                                             start=True, stop=False,
                                             skip_group_check=True,
                                             tile_position=(tp0, 32 * s))
                        nc.scalar.activation(Z[:, qc], zt[:, 0:GSZ],
                                             AF.Sigmoid, bias=BZ)
                        rq = rpool.tile([128, GSZ], bf, tag="rq")
                        nc.scalar.activation(rq[:], rt[:, 0:GSZ],
                                             AF.Sigmoid, bias=BR)
                        rhq = rpool.tile([128, GSZ], bf, tag="rhq")
                        nc.vector.tensor_tensor(out=rhq[:], in0=rq[:],
                                                in1=H[:, qc], op=ALU.mult)
                        for s in range(4):
                            r0 = slice(32 * s, 32 * s + 32)
                            ws = slice((s * 3 + 2) * 32, (s * 3 + 3) * 32)
                            nc.tensor.matmul(hcp[r0, 0:GSZ],
                                             lhsT=W2[:, ws],
                                             rhs=rhq[:, :],
                                             start=False, stop=True,
                                             skip_group_check=True,
                                             tile_position=(0, 32 * s))
                        nc.scalar.activation(HC[:, qc], hcp[:, 0:GSZ],
                                             AF.Tanh, bias=BH)
                    # h update over this quarter's nodes
                    nc.vector.tensor_tensor(out=T1[:, qcols], in0=H[:, qcols],
                                            in1=HC[:, qcols], op=ALU.subtract)
                    nc.vector.tensor_tensor(out=T2[:, qcols], in0=Z[:, qcols],
                                            in1=T1[:, qcols], op=ALU.mult)
                    nc.vector.tensor_tensor(out=H[:, qcols], in0=HC[:, qcols],
                                            in1=T2[:, qcols], op=ALU.add)
                    nc.vector.tensor_scalar(out=T1[:, qcols], in0=H[:, qcols],
                                            scalar1=float(probs[t]),
                                            scalar2=None, op0=ALU.mult)
                    nc.vector.tensor_tensor(out=ACC[:, qcols],
                                            in0=ACC[:, qcols],
                                            in1=T1[:, qcols], op=ALU.add)
                # ---- head for this quarter
                if phases >= 3:
                    nc.vector.tensor_scalar(out=T1[:, qcols],
                                            in0=ACC[:, qcols], scalar1=0.0,
                                            scalar2=None, op0=ALU.max)
                    ncols = NQ * GSZ
                    for s in range(4):
                        for c0 in range(2 * Q * GSZ, (2 * Q + 2) * GSZ, 512):
                            cw = min(512, (2 * Q + 2) * GSZ - c0)
                            hp = hcpsp.tile([128, 512], f32, space="PSUM",
                                            tag="hc")
                            nc.tensor.matmul(hp[0:1, 0:cw],
                                             lhsT=HD[:, s:s + 1],
                                             rhs=T1[:, c0:c0 + cw],
                                             start=True, stop=True,
                                             skip_group_check=True,
                                             tile_position=(0, 0))
                            ot = opool.tile([1, 512], bf, tag="ot")
                            nc.scalar.activation(ot[0:1, 0:cw],
                                                 hp[0:1, 0:cw],
                                                 AF.Sigmoid,
                                                 bias=float(head_bd))
                            nc.sync.dma_start(
                                out.ap()[0:1,
                                         s * ncols + c0:s * ncols + c0 + cw],
                                ot[0:1, 0:cw])

            # quarter boundaries: emit phase-2 quarter Q once all windows
            # covering nodes < (Q+1)*2*4*GSZ are flushed
            qbound = {}
            for Q in range(4):
                hi_node = (Q + 1) * 2 * 4 * GSZ
                b_needed = min(cfg.NB - 1,
                               math.ceil(hi_node / (cfg.W * cfg.B)) - 1)
                if Q == 3:
                    b_needed = cfg.NB - 1
                qbound.setdefault(b_needed, []).append(Q)

            for b in range(cfg.NB):
                gs = list(range(b * cfg.B, min((b + 1) * cfg.B, cfg.NW)))
                # one window per PSUM tile (start=True zeroes the whole
                # 2KB zero region, so windows must not share a bank)
                wtiles = {}
                for g in gs:
                    pt = wpsp.tile([128, 192], f32, tag="wps")
                    wtiles[g] = (pt, 0)
                started = set()
                ends = {g: sum(int(nchunks[g * cfg.KCH + kk])
                               for kk in range(cfg.KCH)) for g in gs}
                done = {g: 0 for g in gs}

                # batched one-hot S tiles for this batch's chunk columns
                c_lo, c_hi = batch_cols[b]
                sb_tiles = {}
                for c0 in range(c_lo, c_hi, 16):
                    cnt = min(16, c_hi - c0)
                    sb = spool.tile([128, 16, 128], f8, tag="sb")
                    dv = DSTL[:, c0:c0 + cnt]
                    dvb = AP(dv.tensor, dv.offset,
                             [list(dv.ap[0]), list(dv.ap[1]), [0, 128]])
                    nc.vector.tensor_tensor(out=sb[:, 0:cnt, :],
                                            in0=IOTA[:, 0:cnt, :],
                                            in1=dvb, op=ALU.is_equal)
                    sb_tiles[c0] = sb

                for (bb, k, nch, wins, coff, ioff) in [e for e in bk_chunks
                                                       if e[0] == b]:
                    nidx = nch * 128
                    it = ipool.tile([128, nidx // 16], i16, tag="idx")
                    nc.sync.dma_start(it[:], idxw.ap()[:, ioff // 16:
                                                       (ioff + nidx) // 16])
                    gt = gpool.tile([128, maxnc, cfg.FPAD], f8, tag="g")
                    nc.gpsimd.dma_gather(
                        out_ap=gt[:, 0:nch, :],
                        in_ap=xap[k * cfg.CHROWS:(k + 1) * cfg.CHROWS, :],
                        idxs_ap=it[:],
                        num_idxs=nidx,
                        num_idxs_reg=nidx,
                        elem_size=cfg.FPAD,
                        single_packet=False,
                    )
                    ci = 0
                    for (g, nchw) in wins:
                        pt, po = wtiles[g]
                        for _ in range(nchw):
                            cc = coff + ci
                            sb = sb_tiles[c_lo + ((cc - c_lo) // 16) * 16]
                            S = sb[:, (cc - c_lo) % 16, :]
                            done[g] += 1
                            nc.tensor.matmul(
                                out=pt[:, po:po + 192],
                                lhsT=S,
                                rhs=gt[:, ci, 0:192],
                                start=(g not in started),
                                stop=(done[g] == ends[g]),
                            )
                            started.add(g)
                            ci += 1

                # flush: scale by dinv, transpose into A_t/B_t
                for g in gs:
                    pt, po = wtiles[g]
                    ys = ypool.tile([128, 192], bf, tag="y")
                    nc.scalar.activation(ys[:], pt[:, po:po + 192], AF.Copy,
                                         scale=DINV[:, g:g + 1])
                    tt = tpsp.tile([128, 256], bf, space="PSUM", tag="tt")
                    nc.tensor.transpose(tt[:, 0:128], ys[:, 0:128], IDN)
                    nc.tensor.transpose(tt[0:64, 128:256], ys[:, 128:192],
                                        IDN)
                    c0 = g * cfg.W
                    nc.vector.tensor_copy(A_t[:, c0:c0 + 128], tt[:, 0:128])
                    nc.vector.tensor_copy(B_t[:, c0:c0 + 128],
                                          tt[0:64, 128:256])

                for Q in qbound.get(b, []):
                    emit_phase2_quarter(Q)

            if phases == 1:
                nc.sync.dma_start(dbg.ap(), A_t)
                nc.sync.dma_start(out.ap(), A_t[0:1, :])
            if phases == 2:
                nc.sync.dma_start(dbg.ap()[:, 0:NQ * GSZ], ACC)
                nc.sync.dma_start(out.ap(), A_t[0:1, :])
            p1.close()

    nc.compile()
    return nc


# ---------------------------------------------------------------- host data

def node_col_of(cfg):
    """Map node id -> output column (block layout: node block
    i = 8*(q//2) + (q%2) + 2*s  <->  out col s*NQ*GSZ + q*GSZ + off)."""
    n = np.arange(cfg.NODE_PAD)
    blk = n // cfg.GSZ
    off = n % cfg.GSZ
    Qq = blk // 8
    i0 = blk % 8
    q = 2 * Qq + (i0 % 2)
    s = i0 // 2
    return s * (cfg.NQ * cfg.GSZ) + q * cfg.GSZ + off


def make_inputs(cfg, x, edge_index, attention,
                conv_wz, conv_bz, conv_wr, conv_br, conv_wh, conv_bh,
                lin_wz, lin_bz, lin_wr, lin_br, lin_wh, lin_bh,
                head_w, head_b):
    """Plan + build all per-core input arrays. Returns (shared, in_maps,
    probs, head_bd)."""
    n = cfg.N
    src = np.asarray(edge_index[0], dtype=np.int64)
    dst = np.asarray(edge_index[1], dtype=np.int64)
    deg = np.bincount(dst, minlength=n).astype(np.float64) + 1.0
    dinv = (1.0 / np.sqrt(deg)).astype(np.float32)

    shared, idx16_all, dstl_all = plan(cfg, edge_index)

    # prescaled, t-major, padded features (fp8 e4m3)
    xt = np.asarray(x, dtype=np.float32)
    xtm = np.transpose(xt, (0, 2, 1)).reshape(n, F_IN * T)  # col t*16+f
    xtm = xtm * dinv[:, None]
    xp = np.zeros((n, cfg.FPAD), dtype=F8NP)
    xp[:, :cfg.FP] = xtm.astype(F8NP)

    # folded GRU weights
    W1g = [np.asarray(conv_wz) @ np.asarray(lin_wz)[:HID],
           np.asarray(conv_wr) @ np.asarray(lin_wr)[:HID],
           np.asarray(conv_wh) @ np.asarray(lin_wh)[:HID]]   # [16,32] each
    W2g = [np.asarray(lin_wz)[HID:], np.asarray(lin_wr)[HID:],
           np.asarray(lin_wh)[HID:]]                          # [32,32] each
    bg = [np.asarray(conv_bz) @ np.asarray(lin_wz)[:HID] + np.asarray(lin_bz),
          np.asarray(conv_br) @ np.asarray(lin_wr)[:HID] + np.asarray(lin_br),
          np.asarray(conv_bh) @ np.asarray(lin_wh)[:HID] + np.asarray(lin_bh)]

    # W1big: per (t, gate) a [128, 32] column block; only the 16 rows of
    # timestep t's feature slice (within A_t / B_t) are nonzero.
    w1big = np.zeros((128, 12 * 3 * 32), dtype=np.float32)
    for t in range(T):
        tt = t if t < 8 else t - 8
        r = 32 * (tt // 2) + 16 * (tt % 2)
        for gate in range(3):
            w1big[r:r + 16, (t * 3 + gate) * 32:(t * 3 + gate + 1) * 32] = \
                W1g[gate]
    # W2big: per (s, gate) a [128, 32] block nonzero only at rows 32s..
    w2big = np.zeros((128, 4 * 3 * 32), dtype=np.float32)
    for s in range(4):
        for gate in range(3):
            w2big[32 * s:32 * s + 32,
                  (s * 3 + gate) * 32:(s * 3 + gate + 1) * 32] = W2g[gate]
    bz = np.tile(bg[0], 4)[:, None].astype(np.float32)
    br = np.tile(bg[1], 4)[:, None].astype(np.float32)
    bh = np.tile(bg[2], 4)[:, None].astype(np.float32)

    hw = np.asarray(head_w, dtype=np.float32)
    hb = np.asarray(head_b, dtype=np.float32)
    hd = np.zeros((128, 4), dtype=np.float32)
    for s in range(4):
        hd[32 * s:32 * s + 32, s] = hw[:, 1] - hw[:, 0]
    head_bd = float(hb[1] - hb[0])

    a = np.asarray(attention, dtype=np.float64)
    e = np.exp(a - a.max())
    probs = (e / e.sum()).astype(np.float32)

    iota = np.broadcast_to(np.tile(np.arange(128, dtype=np.float32), 16),
                           (128, 16 * 128))
    ident = np.eye(128, dtype=np.float32)

    # per-core dinv layout [128, NW]
    in_maps = []
    for c in range(NCORES):
        dv = np.zeros((128, cfg.NW), dtype=np.float32)
        for g in range(cfg.NW):
            lo = g * cfg.W
            hi = min(lo + cfg.W, cfg.NLOC)
            if hi > lo:
                dv[0:hi - lo, g] = dinv[c * cfg.NLOC + lo:c * cfg.NLOC + hi]
        idxc = idx16_all[c]
        idxwr = np.tile(np.ascontiguousarray(idxc.reshape(-1, 16).T), (8, 1))
        dstlc = np.ascontiguousarray(
            dstl_all[c].reshape(-1, 128).T).astype(BF16)
        in_maps.append({
            "xp": xp,
            "idxw": np.ascontiguousarray(idxwr),
            "dstl": dstlc,
            "dinv": dv,
            "w1big": w1big.astype(BF16),
            "w2big": w2big.astype(BF16),
            "bz": bz, "br": br, "bh": bh,
            "hd": hd.astype(BF16),
            "iota": np.ascontiguousarray(iota).astype(BF16),
            "ident": ident.astype(BF16),
        })
    return shared, in_maps, probs, head_bd


_CACHE = {}


def _get_program(cfg, shared, probs, head_bd):
    key = ("v2", cfg.N, shared["tot"],
           tuple(np.asarray(probs).tolist()), head_bd)
    if key not in _CACHE:
        _CACHE[key] = build_program(cfg, shared, probs, head_bd)
    return _CACHE[key]


def kernel(x, edge_index, attention,
           conv_wz, conv_bz, conv_wr, conv_br, conv_wh, conv_bh,
           lin_wz, lin_bz, lin_wr, lin_br, lin_wh, lin_bh,
           head_w, head_b, _trace=False, _cfg=None):
    from concourse import bass_utils

    cfg = _cfg or FULL
    shared, in_maps, probs, head_bd = make_inputs(
        cfg, x, edge_index, attention,
        conv_wz, conv_bz, conv_wr, conv_br, conv_wh, conv_bh,
        lin_wz, lin_bz, lin_wr, lin_br, lin_wh, lin_bh, head_w, head_b)
    nc = _get_program(cfg, shared, probs, head_bd)
    res = bass_utils.run_bass_kernel_spmd(
        nc, in_maps, core_ids=list(range(NCORES)), trace=_trace)

    colmap = node_col_of(cfg)[:cfg.NLOC]
    p1 = np.concatenate(
        [np.asarray(r["out"][0], dtype=np.float32)[colmap]
         for r in res.results])
    outp = np.empty((cfg.N, N_CLS), dtype=np.float32)
    outp[:, 1] = p1
    outp[:, 0] = 1.0 - p1
    if _trace:
        return outp, res
    return outp


# revision 7
# speedup vs baseline: 1.2176x; 1.2176x over previous
"""AttentionGNN (A3TGCN) Trainium2 kernel — self-contained.

Math restructuring (exact):
  GCNConv is linear and A_hat = D^-1/2 (A+I) D^-1/2 is fixed across the 12
  timesteps and 3 gates, so the sparse aggregation is done ONCE on the
  stacked features X' = dinv * x  (shape [N, 192], t-major columns):
      Y = dinv * ((A+I) @ X')
  Everything downstream (GRU cell per timestep, attention sum, head) is
  dense [N,32] work.

Distribution: nodes are partitioned into 8 contiguous dst-ranges (one per
NeuronCore).  Every core receives the full prescaled X' in HBM as fp8
(e4m3, 256B rows) and gathers the rows for its ~400k incoming edges with
dma_gather, reducing them per 127-node dst windows via one-hot matmuls
(fp8) into PSUM.  One-hot scatter matrices are built 16 chunks at a time
with a single broadcast-AP is_equal on the vector engine.  The GRU time
loop runs on the transposed [feat, node] layout, split into 4
node-contiguous quarters so it pipelines into the gather shadow.

Host-side work is limited to integer planning (bucketing / padding edge
lists) and the D^-1/2 prescale; all O(E*F) float work runs on the
NeuronCores.
"""

import math
from contextlib import ExitStack

import numpy as np

try:
    import ml_dtypes

    BF16 = ml_dtypes.bfloat16
    F8NP = ml_dtypes.float8_e4m3fn
except ImportError:  # pragma: no cover
    BF16 = None
    F8NP = None

N_GLOBAL = 100_000
F_IN = 16
T = 12
HID = 32
N_CLS = 2
NCORES = 8


class Cfg:
    def __init__(self, n, b_windows=4, gsz=400, nq=8):
        assert n % NCORES == 0
        self.N = n
        self.NLOC = n // NCORES
        self.W = 127                      # real dst nodes per window
        self.NW = math.ceil(self.NLOC / self.W)
        self.B = b_windows                # windows per PSUM batch
        self.NB = math.ceil(self.NW / self.B)
        self.KCH = 4                      # src chunks (int16 gather indices)
        self.CHROWS = math.ceil(n / self.KCH)
        assert self.CHROWS <= 32767
        self.GSZ = gsz                    # GRU group size (nodes)
        self.NQ = nq                      # GRU quads (groups = 4*NQ)
        self.NODE_PAD = 4 * nq * gsz
        assert self.NODE_PAD >= self.W * self.NW + 1
        self.FP = 192                     # real feature columns (t*16+f)
        self.FPAD = 256                   # padded row length (256B fp8)


FULL = Cfg(N_GLOBAL, b_windows=4, gsz=400, nq=8)


# ---------------------------------------------------------------- planning

def plan(cfg, edge_index):
    """Bucket edges (plus self loops) per core into (window g, src-chunk k)
    buckets, pad each bucket to a multiple of 128 slots that is uniform
    across cores, and emit flat idx16 / dstl arrays in processing order:
       for batch b: for k: for window g in batch: bucket slots.
    Returns (shared structure, per-core arrays)."""
    src = np.concatenate([edge_index[0], np.arange(cfg.N, dtype=np.int64)])
    dst = np.concatenate([edge_index[1], np.arange(cfg.N, dtype=np.int64)])
    src = src.astype(np.int64)
    dst = dst.astype(np.int64)

    core = dst // cfg.NLOC
    NWK = cfg.NW * cfg.KCH
    counts = np.zeros((NCORES, NWK), dtype=np.int64)
    per_core = []
    for c in range(NCORES):
        m = core == c
        s = src[m]
        d = dst[m] - c * cfg.NLOC
        g = d // cfg.W
        dstl = (d - g * cfg.W).astype(np.int64)
        k = s // cfg.CHROWS
        i16 = (s - k * cfg.CHROWS).astype(np.int64)
        bidx = g // cfg.B
        order = np.lexsort((g, k, bidx))
        g, dstl, k, i16 = g[order], dstl[order], k[order], i16[order]
        bucket = g * cfg.KCH + k
        counts[c] = np.bincount(bucket, minlength=NWK)
        per_core.append((bucket, dstl, i16))

    maxcnt = counts.max(axis=0)
    nchunks = -(-maxcnt // 128)           # ceil, per (g,k) flattened g*KCH+k
    assert (nchunks > 0).all()
    slots = nchunks * 128

    # bucket processing order and offsets
    order_buckets = []                    # flattened (g*KCH+k) in emit order
    for b in range(cfg.NB):
        gs = range(b * cfg.B, min((b + 1) * cfg.B, cfg.NW))
        for k in range(cfg.KCH):
            for g in gs:
                order_buckets.append(g * cfg.KCH + k)
    order_buckets = np.array(order_buckets, dtype=np.int64)
    off_in_order = np.zeros(NWK, dtype=np.int64)
    off_in_order[order_buckets] = np.concatenate(
        [[0], np.cumsum(slots[order_buckets])[:-1]]
    )
    tot = int(slots.sum())

    idx16_all = np.zeros((NCORES, tot), dtype=np.int16)
    dstl_all = np.full((NCORES, tot), cfg.W, dtype=np.int16)  # pad -> dead row
    for c in range(NCORES):
        bucket, dstl, i16 = per_core[c]
        # rank within bucket (buckets appear as contiguous runs after sort,
        # but NOT in bucket-id order — use run change points)
        chg = np.flatnonzero(np.r_[True, bucket[1:] != bucket[:-1]])
        run_start_pos = np.repeat(chg, np.diff(np.r_[chg, bucket.size]))
        rank = np.arange(bucket.size, dtype=np.int64) - run_start_pos
        pos = off_in_order[bucket] + rank
        idx16_all[c, pos] = i16.astype(np.int16)
        dstl_all[c, pos] = dstl.astype(np.int16)

    shared = {
        "nchunks": nchunks,               # [NW*KCH]
        "tot": tot,
        "order_buckets": order_buckets,
        "off": off_in_order,
    }
    return shared, idx16_all, dstl_all


# ---------------------------------------------------------------- builder

def build_program(cfg, shared, probs, head_bd, phases=3):
    """Build the SPMD Bacc program (identical for all cores)."""
    from concourse import bacc, mybir
    from concourse.bass import AP
    import concourse.tile as tile

    bf = mybir.dt.bfloat16
    f8 = mybir.dt.float8e4
    f32 = mybir.dt.float32
    i16 = mybir.dt.int16
    AF = mybir.ActivationFunctionType
    ALU = mybir.AluOpType

    nchunks = shared["nchunks"]
    tot = shared["tot"]

    nc = bacc.Bacc("TRN2", target_bir_lowering=False, debug=False,
                   enable_asserts=False, num_devices=NCORES)

    # ---- DRAM I/O
    xp = nc.dram_tensor("xp", [cfg.N, cfg.FPAD], bf, kind="ExternalInput")
    idxw = nc.dram_tensor("idxw", [128, tot // 16], i16, kind="ExternalInput")
    dstl = nc.dram_tensor("dstl", [128, tot // 128], bf, kind="ExternalInput")
    dinv = nc.dram_tensor("dinv", [128, cfg.NW], f32, kind="ExternalInput")
    w1 = nc.dram_tensor("w1big", [128, 1152], bf, kind="ExternalInput")
    w2 = nc.dram_tensor("w2big", [128, 384], bf, kind="ExternalInput")
    bzd = nc.dram_tensor("bz", [128, 1], f32, kind="ExternalInput")
    brd = nc.dram_tensor("br", [128, 1], f32, kind="ExternalInput")
    bhd = nc.dram_tensor("bh", [128, 1], f32, kind="ExternalInput")
    hdd = nc.dram_tensor("hd", [128, 4], bf, kind="ExternalInput")
    iod = nc.dram_tensor("iota", [128, 16 * 128], bf, kind="ExternalInput")
    idd = nc.dram_tensor("ident", [128, 128], bf, kind="ExternalInput")
    out = nc.dram_tensor("out", [1, cfg.NODE_PAD], bf, kind="ExternalOutput")
    dbg = (nc.dram_tensor("dbg", [128, cfg.NODE_PAD], bf, kind="ExternalOutput")
           if phases < 3 else None)

    NP = cfg.NODE_PAD
    GSZ, NQ = cfg.GSZ, cfg.NQ

    # ---- persistent SBUF tensors
    A_t = nc.alloc_sbuf_tensor("A_t", [128, NP], bf).ap()     # t0..t7 feats
    B_t = nc.alloc_sbuf_tensor("B_t", [64, NP], bf).ap()      # t8..t11 feats
    H = nc.alloc_sbuf_tensor("H", [128, NQ * GSZ], bf).ap()
    Z = nc.alloc_sbuf_tensor("Z", [128, NQ * GSZ], bf).ap()
    HC = nc.alloc_sbuf_tensor("HC", [128, NQ * GSZ], bf).ap()
    ACC = nc.alloc_sbuf_tensor("ACCt", [128, NQ * GSZ], bf).ap()
    T1 = nc.alloc_sbuf_tensor("T1", [128, NQ * GSZ], bf).ap()
    T2 = nc.alloc_sbuf_tensor("T2", [128, NQ * GSZ], bf).ap()
    W1 = nc.alloc_sbuf_tensor("W1", [128, 1152], bf).ap()
    W2 = nc.alloc_sbuf_tensor("W2", [128, 384], bf).ap()
    BZ = nc.alloc_sbuf_tensor("BZ", [128, 1], f32).ap()
    BR = nc.alloc_sbuf_tensor("BR", [128, 1], f32).ap()
    BH = nc.alloc_sbuf_tensor("BH", [128, 1], f32).ap()
    HD = nc.alloc_sbuf_tensor("HD", [128, 4], bf).ap()
    IOTA = nc.alloc_sbuf_tensor("IOTA", [128, 16, 128], bf).ap()
    IDN = nc.alloc_sbuf_tensor("IDN", [128, 128], bf).ap()
    DINV = nc.alloc_sbuf_tensor("DINV", [128, cfg.NW], f32).ap()
    DSTL = nc.alloc_sbuf_tensor("DSTL", [128, tot // 128], bf).ap()

    # gather tile sizing: max chunks for one (batch, k) gather
    bk_chunks = []                        # [(b, k, nidx, [(g, nch), ...])]
    coff = 0
    ioff = 0
    for b in range(cfg.NB):
        gs = list(range(b * cfg.B, min((b + 1) * cfg.B, cfg.NW)))
        for k in range(cfg.KCH):
            wins = [(g, int(nchunks[g * cfg.KCH + k])) for g in gs]
            nch = sum(w[1] for w in wins)
            bk_chunks.append((b, k, nch, wins, coff, ioff))
            coff += nch
            ioff += nch * 128
    maxnc = max(e[2] for e in bk_chunks)

    # batch -> (chunk col range) for one-hot building
    batch_cols = {}
    for (b, k, nch, wins, coff_, ioff_) in bk_chunks:
        lo, hi = batch_cols.get(b, (coff_, coff_ + nch))
        batch_cols[b] = (min(lo, coff_), max(hi, coff_ + nch))

    with tile.TileContext(nc) as tc:
        with tc.tile_pool(name="cpool", bufs=1) as cpool:
            # ---- load constants
            nc.sync.dma_start(W1, w1.ap())
            nc.sync.dma_start(W2, w2.ap())
            nc.sync.dma_start(BZ, bzd.ap())
            nc.sync.dma_start(BR, brd.ap())
            nc.sync.dma_start(BH, bhd.ap())
            nc.sync.dma_start(HD, hdd.ap())
            nc.sync.dma_start(IOTA[:, :, :],
                              AP(iod, 0, [[iod.ap().ap[0][0], 128],
                                          [128, 16], [1, 128]]))
            nc.sync.dma_start(IDN, idd.ap())
            nc.sync.dma_start(DINV, dinv.ap())
            nc.sync.dma_start(DSTL, dstl.ap())
            nc.gpsimd.memset(H, 0)
            nc.gpsimd.memset(ACC, 0)
            tail = cfg.W * cfg.NW + 1
            if tail < NP:
                nc.gpsimd.memset(A_t[:, tail - 128:NP], 0)
                nc.gpsimd.memset(B_t[:, tail - 128:NP], 0)

            # ================= phase 1: aggregation =================
            xap = xp.ap()
            p1 = ExitStack()
            gpool = p1.enter_context(tc.tile_pool(name="gpool", bufs=3))
            ipool = p1.enter_context(tc.tile_pool(name="ipool", bufs=3))
            spool = p1.enter_context(tc.tile_pool(name="spool", bufs=4))
            ypool = p1.enter_context(tc.tile_pool(name="ypool", bufs=3))
            wpsp = p1.enter_context(
                tc.tile_pool(name="wps", bufs=4, space="PSUM"))
            tpsp = p1.enter_context(
                tc.tile_pool(name="tps", bufs=1, space="PSUM"))

            # phase-2 pools (allocated up front; used interleaved per quarter)
            rpool = p1.enter_context(tc.tile_pool(name="rpool", bufs=2))
            opool = p1.enter_context(tc.tile_pool(name="opool", bufs=2))
            zrpsp = p1.enter_context(
                tc.tile_pool(name="zrps", bufs=1, space="PSUM"))
            hcpsp = p1.enter_context(
                tc.tile_pool(name="hcps", bufs=1, space="PSUM"))

            def emit_phase2_quarter(Q):
                """GRU + attention + head for quads 2Q, 2Q+1 (nodes
                [Q*2*4*GSZ, (Q+1)*2*4*GSZ) in block layout)."""
                if phases < 2:
                    return
                qcols = slice(2 * Q * GSZ, (2 * Q + 2) * GSZ)
                for t in range(T):
                    Ysrc = A_t if t < 8 else B_t
                    KH = 128 if t < 8 else 64
                    tp0 = 0
                    wz = W1[0:KH, (t * 3 + 0) * 32:(t * 3 + 1) * 32]
                    wr = W1[0:KH, (t * 3 + 1) * 32:(t * 3 + 2) * 32]
                    wh = W1[0:KH, (t * 3 + 2) * 32:(t * 3 + 3) * 32]
                    for q in (2 * Q, 2 * Q + 1):
                        qc = slice(q * GSZ, (q + 1) * GSZ)
                        zt = zrpsp.tile([128, 512], f32, space="PSUM",
                                        tag="zt")
                        rt = zrpsp.tile([128, 512], f32, space="PSUM",
                                        tag="rt")
                        hcp = hcpsp.tile([128, 512], f32, space="PSUM",
                                         tag="hc")
                        for s in range(4):
                            # block layout: quad q, band s -> node block
                            # 8*(q//2) + (q%2) + 2*s
                            blk = 8 * (q // 2) + (q % 2) + 2 * s
                            yv = Ysrc[0:KH, blk * GSZ:(blk + 1) * GSZ]
                            r0 = slice(32 * s, 32 * s + 32)
                            ws = slice((s * 3) * 32, (s * 3 + 1) * 32)
                            nc.tensor.matmul(zt[r0, 0:GSZ], lhsT=wz, rhs=yv,
                                             start=True, stop=False,
                                             skip_group_check=True,
                                             tile_position=(tp0, 32 * s))
                            nc.tensor.matmul(zt[r0, 0:GSZ],
                                             lhsT=W2[:, ws],
                                             rhs=H[:, qc],
                                             start=False, stop=True,
                                             skip_group_check=True,
                                             tile_position=(0, 32 * s))
                            ws = slice((s * 3 + 1) * 32, (s * 3 + 2) * 32)
                            nc.tensor.matmul(rt[r0, 0:GSZ], lhsT=wr,
                                             rhs=yv, start=True, stop=False,
                                             skip_group_check=True,
                                             tile_position=(tp0, 32 * s))
                            nc.tensor.matmul(rt[r0, 0:GSZ],
                                             lhsT=W2[:, ws],
                                             rhs=H[:, qc],
                                             start=False, stop=True,
                                             skip_group_check=True,
                                             tile_position=(0, 32 * s))
                            nc.tensor.matmul(hcp[r0, 0:GSZ], lhsT=wh, rhs=yv,
                                             start=True, stop=False,
                                             skip_group_check=True,
                                             tile_position=(tp0, 32 * s))
                        nc.scalar.activation(Z[:, qc], zt[:, 0:GSZ],
                                             AF.Sigmoid, bias=BZ)
                        rq = rpool.tile([128, GSZ], bf, tag="rq")
                        nc.scalar.activation(rq[:], rt[:, 0:GSZ],
                                             AF.Sigmoid, bias=BR)
                        rhq = rpool.tile([128, GSZ], bf, tag="rhq")
                        nc.vector.tensor_tensor(out=rhq[:], in0=rq[:],
                                                in1=H[:, qc], op=ALU.mult)
                        for s in range(4):
                            r0 = slice(32 * s, 32 * s + 32)
                            ws = slice((s * 3 + 2) * 32, (s * 3 + 3) * 32)
                            nc.tensor.matmul(hcp[r0, 0:GSZ],
                                             lhsT=W2[:, ws],
                                             rhs=rhq[:, :],
                                             start=False, stop=True,
                                             skip_group_check=True,
                                             tile_position=(0, 32 * s))
                        nc.scalar.activation(HC[:, qc], hcp[:, 0:GSZ],
                                             AF.Tanh, bias=BH)
                    # h update over this quarter's nodes
                    nc.vector.tensor_tensor(out=T1[:, qcols], in0=H[:, qcols],
                                            in1=HC[:, qcols], op=ALU.subtract)
                    nc.vector.tensor_tensor(out=T2[:, qcols], in0=Z[:, qcols],
                                            in1=T1[:, qcols], op=ALU.mult)
                    nc.vector.tensor_tensor(out=H[:, qcols], in0=HC[:, qcols],
                                            in1=T2[:, qcols], op=ALU.add)
                    nc.vector.tensor_scalar(out=T1[:, qcols], in0=H[:, qcols],
                                            scalar1=float(probs[t]),
                                            scalar2=None, op0=ALU.mult)
                    nc.vector.tensor_tensor(out=ACC[:, qcols],
                                            in0=ACC[:, qcols],
                                            in1=T1[:, qcols], op=ALU.add)
                # ---- head for this quarter
                if phases >= 3:
                    nc.vector.tensor_scalar(out=T1[:, qcols],
                                            in0=ACC[:, qcols], scalar1=0.0,
                                            scalar2=None, op0=ALU.max)
                    ncols = NQ * GSZ
                    for s in range(4):
                        for c0 in range(2 * Q * GSZ, (2 * Q + 2) * GSZ, 512):
                            cw = min(512, (2 * Q + 2) * GSZ - c0)
                            hp = hcpsp.tile([128, 512], f32, space="PSUM",
                                            tag="hc")
                            nc.tensor.matmul(hp[0:1, 0:cw],
                                             lhsT=HD[:, s:s + 1],
                                             rhs=T1[:, c0:c0 + cw],
                                             start=True, stop=True,
                                             skip_group_check=True,
                                             tile_position=(0, 0))
                            ot = opool.tile([1, 512], bf, tag="ot")
                            nc.scalar.activation(ot[0:1, 0:cw],
                                                 hp[0:1, 0:cw],
                                                 AF.Sigmoid,
                                                 bias=float(head_bd))
                            nc.sync.dma_start(
                                out.ap()[0:1,
                                         s * ncols + c0:s * ncols + c0 + cw],
                                ot[0:1, 0:cw])

            # quarter boundaries: emit phase-2 quarter Q once all windows
            # covering nodes < (Q+1)*2*4*GSZ are flushed
            qbound = {}
            for Q in range(4):
                hi_node = (Q + 1) * 2 * 4 * GSZ
                b_needed = min(cfg.NB - 1,
                               math.ceil(hi_node / (cfg.W * cfg.B)) - 1)
                if Q == 3:
                    b_needed = cfg.NB - 1
                qbound.setdefault(b_needed, []).append(Q)

            for b in range(cfg.NB):
                gs = list(range(b * cfg.B, min((b + 1) * cfg.B, cfg.NW)))
                # one window per PSUM tile (start=True zeroes the whole
                # 2KB zero region, so windows must not share a bank)
                wtiles = {}
                for g in gs:
                    pt = wpsp.tile([128, 192], f32, tag="wps")
                    wtiles[g] = (pt, 0)
                started = set()
                ends = {g: sum(int(nchunks[g * cfg.KCH + kk])
                               for kk in range(cfg.KCH)) for g in gs}
                done = {g: 0 for g in gs}

                # batched one-hot S tiles for this batch's chunk columns
                c_lo, c_hi = batch_cols[b]
                sb_tiles = {}
                for c0 in range(c_lo, c_hi, 16):
                    cnt = min(16, c_hi - c0)
                    sb = spool.tile([128, 16, 128], bf, tag="sb")
                    dv = DSTL[:, c0:c0 + cnt]
                    dvb = AP(dv.tensor, dv.offset,
                             [list(dv.ap[0]), list(dv.ap[1]), [0, 128]])
                    nc.vector.tensor_tensor(out=sb[:, 0:cnt, :],
                                            in0=IOTA[:, 0:cnt, :],
                                            in1=dvb, op=ALU.is_equal)
                    sb_tiles[c0] = sb

                for (bb, k, nch, wins, coff, ioff) in [e for e in bk_chunks
                                                       if e[0] == b]:
                    nidx = nch * 128
                    it = ipool.tile([128, nidx // 16], i16, tag="idx")
                    nc.sync.dma_start(it[:], idxw.ap()[:, ioff // 16:
                                                       (ioff + nidx) // 16])
                    gt = gpool.tile([128, maxnc, cfg.FPAD], bf, tag="g")
                    nc.gpsimd.dma_gather(
                        out_ap=gt[:, 0:nch, :],
                        in_ap=xap[k * cfg.CHROWS:(k + 1) * cfg.CHROWS, :],
                        idxs_ap=it[:],
                        num_idxs=nidx,
                        num_idxs_reg=nidx,
                        elem_size=cfg.FPAD,
                        single_packet=False,
                    )
                    ci = 0
                    for (g, nchw) in wins:
                        pt, po = wtiles[g]
                        for _ in range(nchw):
                            cc = coff + ci
                            sb = sb_tiles[c_lo + ((cc - c_lo) // 16) * 16]
                            S = sb[:, (cc - c_lo) % 16, :]
                            done[g] += 1
                            nc.tensor.matmul(
                                out=pt[:, po:po + 192],
                                lhsT=S,
                                rhs=gt[:, ci, 0:192],
                                start=(g not in started),
                                stop=(done[g] == ends[g]),
                            )
                            started.add(g)
                            ci += 1

                # flush: scale by dinv, transpose into A_t/B_t
                for g in gs:
                    pt, po = wtiles[g]
                    ys = ypool.tile([128, 192], bf, tag="y")
                    nc.scalar.activation(ys[:], pt[:, po:po + 192], AF.Copy,
                                         scale=DINV[:, g:g + 1])
                    tt = tpsp.tile([128, 256], bf, space="PSUM", tag="tt")
                    nc.tensor.transpose(tt[:, 0:128], ys[:, 0:128], IDN)
                    nc.tensor.transpose(tt[0:64, 128:256], ys[:, 128:192],
                                        IDN)
                    c0 = g * cfg.W
                    nc.vector.tensor_copy(A_t[:, c0:c0 + 128], tt[:, 0:128])
                    nc.vector.tensor_copy(B_t[:, c0:c0 + 128],
                                          tt[0:64, 128:256])

                for Q in qbound.get(b, []):
                    emit_phase2_quarter(Q)

            if phases == 1:
                nc.sync.dma_start(dbg.ap(), A_t)
                nc.sync.dma_start(out.ap(), A_t[0:1, :])
            if phases == 2:
                nc.sync.dma_start(dbg.ap()[:, 0:NQ * GSZ], ACC)
                nc.sync.dma_start(out.ap(), A_t[0:1, :])
            p1.close()

    nc.compile()
    return nc


# ---------------------------------------------------------------- host data

def node_col_of(cfg):
    """Map node id -> output column (block layout: node block
    i = 8*(q//2) + (q%2) + 2*s  <->  out col s*NQ*GSZ + q*GSZ + off)."""
    n = np.arange(cfg.NODE_PAD)
    blk = n // cfg.GSZ
    off = n % cfg.GSZ
    Qq = blk // 8
    i0 = blk % 8
    q = 2 * Qq + (i0 % 2)
    s = i0 // 2
    return s * (cfg.NQ * cfg.GSZ) + q * cfg.GSZ + off


def make_inputs(cfg, x, edge_index, attention,
                conv_wz, conv_bz, conv_wr, conv_br, conv_wh, conv_bh,
                lin_wz, lin_bz, lin_wr, lin_br, lin_wh, lin_bh,
                head_w, head_b):
    """Plan + build all per-core input arrays. Returns (shared, in_maps,
    probs, head_bd)."""
    n = cfg.N
    src = np.asarray(edge_index[0], dtype=np.int64)
    dst = np.asarray(edge_index[1], dtype=np.int64)
    deg = np.bincount(dst, minlength=n).astype(np.float64) + 1.0
    dinv = (1.0 / np.sqrt(deg)).astype(np.float32)

    shared, idx16_all, dstl_all = plan(cfg, edge_index)

    # prescaled, t-major, padded features (fp8 e4m3)
    xt = np.asarray(x, dtype=np.float32)
    xtm = np.transpose(xt, (0, 2, 1)).reshape(n, F_IN * T)  # col t*16+f
    xtm = xtm * dinv[:, None]
    xp = np.zeros((n, cfg.FPAD), dtype=BF16)
    xp[:, :cfg.FP] = xtm.astype(BF16)

    # folded GRU weights
    W1g = [np.asarray(conv_wz) @ np.asarray(lin_wz)[:HID],
           np.asarray(conv_wr) @ np.asarray(lin_wr)[:HID],
           np.asarray(conv_wh) @ np.asarray(lin_wh)[:HID]]   # [16,32] each
    W2g = [np.asarray(lin_wz)[HID:], np.asarray(lin_wr)[HID:],
           np.asarray(lin_wh)[HID:]]                          # [32,32] each
    bg = [np.asarray(conv_bz) @ np.asarray(lin_wz)[:HID] + np.asarray(lin_bz),
          np.asarray(conv_br) @ np.asarray(lin_wr)[:HID] + np.asarray(lin_br),
          np.asarray(conv_bh) @ np.asarray(lin_wh)[:HID] + np.asarray(lin_bh)]

    # W1big: per (t, gate) a [128, 32] column block; only the 16 rows of
    # timestep t's feature slice (within A_t / B_t) are nonzero.
    w1big = np.zeros((128, 12 * 3 * 32), dtype=np.float32)
    for t in range(T):
        tt = t if t < 8 else t - 8
        r = 32 * (tt // 2) + 16 * (tt % 2)
        for gate in range(3):
            w1big[r:r + 16, (t * 3 + gate) * 32:(t * 3 + gate + 1) * 32] = \
                W1g[gate]
    # W2big: per (s, gate) a [128, 32] block nonzero only at rows 32s..
    w2big = np.zeros((128, 4 * 3 * 32), dtype=np.float32)
    for s in range(4):
        for gate in range(3):
            w2big[32 * s:32 * s + 32,
                  (s * 3 + gate) * 32:(s * 3 + gate + 1) * 32] = W2g[gate]
    bz = np.tile(bg[0], 4)[:, None].astype(np.float32)
    br = np.tile(bg[1], 4)[:, None].astype(np.float32)
    bh = np.tile(bg[2], 4)[:, None].astype(np.float32)

    hw = np.asarray(head_w, dtype=np.float32)
    hb = np.asarray(head_b, dtype=np.float32)
    hd = np.zeros((128, 4), dtype=np.float32)
    for s in range(4):
        hd[32 * s:32 * s + 32, s] = hw[:, 1] - hw[:, 0]
    head_bd = float(hb[1] - hb[0])

    a = np.asarray(attention, dtype=np.float64)
    e = np.exp(a - a.max())
    probs = (e / e.sum()).astype(np.float32)

    iota = np.broadcast_to(np.tile(np.arange(128, dtype=np.float32), 16),
                           (128, 16 * 128))
    ident = np.eye(128, dtype=np.float32)

    # per-core dinv layout [128, NW]
    in_maps = []
    for c in range(NCORES):
        dv = np.zeros((128, cfg.NW), dtype=np.float32)
        for g in range(cfg.NW):
            lo = g * cfg.W
            hi = min(lo + cfg.W, cfg.NLOC)
            if hi > lo:
                dv[0:hi - lo, g] = dinv[c * cfg.NLOC + lo:c * cfg.NLOC + hi]
        idxc = idx16_all[c]
        idxwr = np.tile(np.ascontiguousarray(idxc.reshape(-1, 16).T), (8, 1))
        dstlc = np.ascontiguousarray(
            dstl_all[c].reshape(-1, 128).T).astype(BF16)
        in_maps.append({
            "xp": xp,
            "idxw": np.ascontiguousarray(idxwr),
            "dstl": dstlc,
            "dinv": dv,
            "w1big": w1big.astype(BF16),
            "w2big": w2big.astype(BF16),
            "bz": bz, "br": br, "bh": bh,
            "hd": hd.astype(BF16),
            "iota": np.ascontiguousarray(iota).astype(BF16),
            "ident": ident.astype(BF16),
        })
    return shared, in_maps, probs, head_bd


_CACHE = {}


def _get_program(cfg, shared, probs, head_bd):
    key = ("v3", cfg.N, shared["tot"],
           tuple(np.asarray(probs).tolist()), head_bd)
    if key not in _CACHE:
        _CACHE[key] = build_program(cfg, shared, probs, head_bd)
    return _CACHE[key]


def kernel(x, edge_index, attention,
           conv_wz, conv_bz, conv_wr, conv_br, conv_wh, conv_bh,
           lin_wz, lin_bz, lin_wr, lin_br, lin_wh, lin_bh,
           head_w, head_b, _trace=False, _cfg=None):
    from concourse import bass_utils

    cfg = _cfg or FULL
    shared, in_maps, probs, head_bd = make_inputs(
        cfg, x, edge_index, attention,
        conv_wz, conv_bz, conv_wr, conv_br, conv_wh, conv_bh,
        lin_wz, lin_bz, lin_wr, lin_br, lin_wh, lin_bh, head_w, head_b)
    nc = _get_program(cfg, shared, probs, head_bd)
    res = bass_utils.run_bass_kernel_spmd(
        nc, in_maps, core_ids=list(range(NCORES)), trace=_trace)

    colmap = node_col_of(cfg)[:cfg.NLOC]
    p1 = np.concatenate(
        [np.asarray(r["out"][0], dtype=np.float32)[colmap]
         for r in res.results])
    outp = np.empty((cfg.N, N_CLS), dtype=np.float32)
    outp[:, 1] = p1
    outp[:, 0] = 1.0 - p1
    if _trace:
        return outp, res
    return outp


# revision 8
# speedup vs baseline: 1.2486x; 1.0255x over previous
"""AttentionGNN (A3TGCN) Trainium2 kernel — self-contained.

Math restructuring (exact):
  GCNConv is linear and A_hat = D^-1/2 (A+I) D^-1/2 is fixed across the 12
  timesteps and 3 gates, so the sparse aggregation is done ONCE on the
  stacked features X' = dinv * x  (shape [N, 192], t-major columns):
      Y = dinv * ((A+I) @ X')
  Everything downstream (GRU cell per timestep, attention sum, head) is
  dense [N,32] work.

Distribution: nodes are partitioned into 8 contiguous dst-ranges (one per
NeuronCore).  Every core receives the full prescaled X' in HBM as fp8
(e4m3, 256B rows) and gathers the rows for its ~400k incoming edges with
dma_gather, reducing them per 127-node dst windows via one-hot matmuls
(fp8) into PSUM.  One-hot scatter matrices are built 16 chunks at a time
with a single broadcast-AP is_equal on the vector engine.  The GRU time
loop runs on the transposed [feat, node] layout, split into 4
node-contiguous quarters so it pipelines into the gather shadow.

Host-side work is limited to integer planning (bucketing / padding edge
lists) and the D^-1/2 prescale; all O(E*F) float work runs on the
NeuronCores.
"""

import math
from contextlib import ExitStack

import numpy as np

try:
    import ml_dtypes

    BF16 = ml_dtypes.bfloat16
    F8NP = ml_dtypes.float8_e4m3fn
except ImportError:  # pragma: no cover
    BF16 = None
    F8NP = None

N_GLOBAL = 100_000
F_IN = 16
T = 12
HID = 32
N_CLS = 2
NCORES = 8


class Cfg:
    def __init__(self, n, b_windows=4, gsz=400, nq=8):
        assert n % NCORES == 0
        self.N = n
        self.NLOC = n // NCORES
        self.W = 127                      # real dst nodes per window
        self.NW = math.ceil(self.NLOC / self.W)
        self.B = b_windows                # windows per PSUM batch
        self.NB = math.ceil(self.NW / self.B)
        self.KCH = 4                      # src chunks (int16 gather indices)
        self.CHROWS = math.ceil(n / self.KCH)
        assert self.CHROWS <= 32767
        self.GSZ = gsz                    # GRU group size (nodes)
        self.NQ = nq                      # GRU quads (groups = 4*NQ)
        self.NODE_PAD = 4 * nq * gsz
        assert self.NODE_PAD >= self.W * self.NW + 1
        self.FP = 192                     # real feature columns (t*16+f)
        self.FPAD = 256                   # padded row length (256B fp8)


FULL = Cfg(N_GLOBAL, b_windows=4, gsz=400, nq=8)


# ---------------------------------------------------------------- planning

def plan(cfg, edge_index):
    """Bucket edges (plus self loops) per core into (window g, src-chunk k)
    buckets, pad each bucket to a multiple of 128 slots that is uniform
    across cores, and emit flat idx16 / dstl arrays in processing order:
       for batch b: for k: for window g in batch: bucket slots.
    Returns (shared structure, per-core arrays)."""
    src = np.asarray(edge_index[0]).astype(np.int64)
    dst = np.asarray(edge_index[1]).astype(np.int64)

    core = dst // cfg.NLOC
    NWK = cfg.NW * cfg.KCH
    counts = np.zeros((NCORES, NWK), dtype=np.int64)
    per_core = []
    for c in range(NCORES):
        m = core == c
        s = src[m]
        d = dst[m] - c * cfg.NLOC
        g = d // cfg.W
        dstl = (d - g * cfg.W).astype(np.int64)
        k = s // cfg.CHROWS
        i16 = (s - k * cfg.CHROWS).astype(np.int64)
        bidx = g // cfg.B
        order = np.lexsort((g, k, bidx))
        g, dstl, k, i16 = g[order], dstl[order], k[order], i16[order]
        bucket = g * cfg.KCH + k
        counts[c] = np.bincount(bucket, minlength=NWK)
        per_core.append((bucket, dstl, i16))

    maxcnt = counts.max(axis=0)
    nchunks = -(-maxcnt // 128)           # ceil, per (g,k) flattened g*KCH+k
    assert (nchunks > 0).all()
    slots = nchunks * 128

    # bucket processing order and offsets
    order_buckets = []                    # flattened (g*KCH+k) in emit order
    for b in range(cfg.NB):
        gs = range(b * cfg.B, min((b + 1) * cfg.B, cfg.NW))
        for k in range(cfg.KCH):
            for g in gs:
                order_buckets.append(g * cfg.KCH + k)
    order_buckets = np.array(order_buckets, dtype=np.int64)
    off_in_order = np.zeros(NWK, dtype=np.int64)
    off_in_order[order_buckets] = np.concatenate(
        [[0], np.cumsum(slots[order_buckets])[:-1]]
    )
    tot = int(slots.sum())

    idx16_all = np.zeros((NCORES, tot), dtype=np.int16)
    dstl_all = np.full((NCORES, tot), cfg.W, dtype=np.int16)  # pad -> dead row
    for c in range(NCORES):
        bucket, dstl, i16 = per_core[c]
        # rank within bucket (buckets appear as contiguous runs after sort,
        # but NOT in bucket-id order — use run change points)
        chg = np.flatnonzero(np.r_[True, bucket[1:] != bucket[:-1]])
        run_start_pos = np.repeat(chg, np.diff(np.r_[chg, bucket.size]))
        rank = np.arange(bucket.size, dtype=np.int64) - run_start_pos
        pos = off_in_order[bucket] + rank
        idx16_all[c, pos] = i16.astype(np.int16)
        dstl_all[c, pos] = dstl.astype(np.int16)

    shared = {
        "nchunks": nchunks,               # [NW*KCH]
        "tot": tot,
        "order_buckets": order_buckets,
        "off": off_in_order,
    }
    return shared, idx16_all, dstl_all


# ---------------------------------------------------------------- builder

def build_program(cfg, shared, probs, head_bd, phases=3):
    """Build the SPMD Bacc program (identical for all cores)."""
    from concourse import bacc, mybir
    from concourse.bass import AP
    import concourse.tile as tile

    bf = mybir.dt.bfloat16
    f8 = mybir.dt.float8e4
    f32 = mybir.dt.float32
    i16 = mybir.dt.int16
    AF = mybir.ActivationFunctionType
    ALU = mybir.AluOpType

    nchunks = shared["nchunks"]
    tot = shared["tot"]

    nc = bacc.Bacc("TRN2", target_bir_lowering=False, debug=False,
                   enable_asserts=False, num_devices=NCORES)

    # ---- DRAM I/O
    xp = nc.dram_tensor("xp", [cfg.N, cfg.FPAD], bf, kind="ExternalInput")
    idxw = nc.dram_tensor("idxw", [128, tot // 16], i16, kind="ExternalInput")
    dstl = nc.dram_tensor("dstl", [128, tot // 128], bf, kind="ExternalInput")
    dinv = nc.dram_tensor("dinv", [128, cfg.NW], f32, kind="ExternalInput")
    w1 = nc.dram_tensor("w1big", [128, 1152], bf, kind="ExternalInput")
    w2 = nc.dram_tensor("w2big", [128, 384], bf, kind="ExternalInput")
    bzd = nc.dram_tensor("bz", [128, 1], f32, kind="ExternalInput")
    brd = nc.dram_tensor("br", [128, 1], f32, kind="ExternalInput")
    bhd = nc.dram_tensor("bh", [128, 1], f32, kind="ExternalInput")
    hdd = nc.dram_tensor("hd", [128, 4], bf, kind="ExternalInput")
    iod = nc.dram_tensor("iota", [128, 16 * 128], bf, kind="ExternalInput")
    idd = nc.dram_tensor("ident", [128, 128], bf, kind="ExternalInput")
    xloc = nc.dram_tensor("xloc", [cfg.NLOC + 128, cfg.FPAD], bf,
                          kind="ExternalInput")
    out = nc.dram_tensor("out", [1, cfg.NODE_PAD], bf, kind="ExternalOutput")
    dbg = (nc.dram_tensor("dbg", [128, cfg.NODE_PAD], bf, kind="ExternalOutput")
           if phases < 3 else None)

    NP = cfg.NODE_PAD
    GSZ, NQ = cfg.GSZ, cfg.NQ

    # ---- persistent SBUF tensors
    A_t = nc.alloc_sbuf_tensor("A_t", [128, NP], bf).ap()     # t0..t7 feats
    B_t = nc.alloc_sbuf_tensor("B_t", [64, NP], bf).ap()      # t8..t11 feats
    H = nc.alloc_sbuf_tensor("H", [128, NQ * GSZ], bf).ap()
    Z = nc.alloc_sbuf_tensor("Z", [128, NQ * GSZ], bf).ap()
    HC = nc.alloc_sbuf_tensor("HC", [128, NQ * GSZ], bf).ap()
    ACC = nc.alloc_sbuf_tensor("ACCt", [128, NQ * GSZ], bf).ap()
    T1 = nc.alloc_sbuf_tensor("T1", [128, NQ * GSZ], bf).ap()
    T2 = nc.alloc_sbuf_tensor("T2", [128, NQ * GSZ], bf).ap()
    W1 = nc.alloc_sbuf_tensor("W1", [128, 1152], bf).ap()
    W2 = nc.alloc_sbuf_tensor("W2", [128, 384], bf).ap()
    BZ = nc.alloc_sbuf_tensor("BZ", [128, 1], f32).ap()
    BR = nc.alloc_sbuf_tensor("BR", [128, 1], f32).ap()
    BH = nc.alloc_sbuf_tensor("BH", [128, 1], f32).ap()
    HD = nc.alloc_sbuf_tensor("HD", [128, 4], bf).ap()
    IOTA = nc.alloc_sbuf_tensor("IOTA", [128, 16, 128], bf).ap()
    IDN = nc.alloc_sbuf_tensor("IDN", [128, 128], bf).ap()
    DINV = nc.alloc_sbuf_tensor("DINV", [128, cfg.NW], f32).ap()
    DSTL = nc.alloc_sbuf_tensor("DSTL", [128, tot // 128], bf).ap()

    # gather tile sizing: max chunks for one (batch, k) gather
    bk_chunks = []                        # [(b, k, nidx, [(g, nch), ...])]
    coff = 0
    ioff = 0
    for b in range(cfg.NB):
        gs = list(range(b * cfg.B, min((b + 1) * cfg.B, cfg.NW)))
        for k in range(cfg.KCH):
            wins = [(g, int(nchunks[g * cfg.KCH + k])) for g in gs]
            nch = sum(w[1] for w in wins)
            bk_chunks.append((b, k, nch, wins, coff, ioff))
            coff += nch
            ioff += nch * 128
    maxnc = max(e[2] for e in bk_chunks)

    # batch -> (chunk col range) for one-hot building
    batch_cols = {}
    for (b, k, nch, wins, coff_, ioff_) in bk_chunks:
        lo, hi = batch_cols.get(b, (coff_, coff_ + nch))
        batch_cols[b] = (min(lo, coff_), max(hi, coff_ + nch))

    with tile.TileContext(nc) as tc:
        with tc.tile_pool(name="cpool", bufs=1) as cpool:
            # ---- load constants
            nc.sync.dma_start(W1, w1.ap())
            nc.sync.dma_start(W2, w2.ap())
            nc.sync.dma_start(BZ, bzd.ap())
            nc.sync.dma_start(BR, brd.ap())
            nc.sync.dma_start(BH, bhd.ap())
            nc.sync.dma_start(HD, hdd.ap())
            nc.sync.dma_start(IOTA[:, :, :],
                              AP(iod, 0, [[iod.ap().ap[0][0], 128],
                                          [128, 16], [1, 128]]))
            nc.sync.dma_start(IDN, idd.ap())
            nc.sync.dma_start(DINV, dinv.ap())
            nc.sync.dma_start(DSTL, dstl.ap())
            nc.vector.memset(H, 0)
            nc.vector.memset(ACC, 0)
            tail = cfg.W * cfg.NW + 1
            if tail < NP:
                nc.vector.memset(A_t[:, tail - 128:NP], 0)
                nc.vector.memset(B_t[:, tail - 128:NP], 0)

            # ================= phase 1: aggregation =================
            xap = xp.ap()
            p1 = ExitStack()
            gpool = p1.enter_context(tc.tile_pool(name="gpool", bufs=3))
            ipool = p1.enter_context(tc.tile_pool(name="ipool", bufs=3))
            spool = p1.enter_context(tc.tile_pool(name="spool", bufs=4))
            ypool = p1.enter_context(tc.tile_pool(name="ypool", bufs=3))
            xlpool = p1.enter_context(tc.tile_pool(name="xlpool", bufs=4))
            wpsp = p1.enter_context(
                tc.tile_pool(name="wps", bufs=4, space="PSUM"))
            tpsp = p1.enter_context(
                tc.tile_pool(name="tps", bufs=1, space="PSUM"))

            # phase-2 pools (allocated up front; used interleaved per quarter)
            rpool = p1.enter_context(tc.tile_pool(name="rpool", bufs=2))
            opool = p1.enter_context(tc.tile_pool(name="opool", bufs=2))
            zrpsp = p1.enter_context(
                tc.tile_pool(name="zrps", bufs=1, space="PSUM"))
            hcpsp = p1.enter_context(
                tc.tile_pool(name="hcps", bufs=1, space="PSUM"))

            def emit_phase2_quarter(Q):
                """GRU + attention + head for quads 2Q, 2Q+1 (nodes
                [Q*2*4*GSZ, (Q+1)*2*4*GSZ) in block layout)."""
                if phases < 2:
                    return
                qcols = slice(Q * 4 * GSZ // 4, 0) if False else slice(Q * GSZ, (Q + 1) * GSZ)
                for t in range(T):
                    Ysrc = A_t if t < 8 else B_t
                    KH = 128 if t < 8 else 64
                    tp0 = 0
                    wz = W1[0:KH, (t * 3 + 0) * 32:(t * 3 + 1) * 32]
                    wr = W1[0:KH, (t * 3 + 1) * 32:(t * 3 + 2) * 32]
                    wh = W1[0:KH, (t * 3 + 2) * 32:(t * 3 + 3) * 32]
                    for q in (Q,):
                        qc = slice(q * GSZ, (q + 1) * GSZ)
                        zt = zrpsp.tile([128, 512], f32, space="PSUM",
                                        tag="zt")
                        rt = zrpsp.tile([128, 512], f32, space="PSUM",
                                        tag="rt")
                        hcp = hcpsp.tile([128, 512], f32, space="PSUM",
                                         tag="hc")
                        for s in range(4):
                            # block layout: quad q, band s -> node block
                            # 8*(q//2) + (q%2) + 2*s
                            blk = 4 * q + s
                            yv = Ysrc[0:KH, blk * GSZ:(blk + 1) * GSZ]
                            r0 = slice(32 * s, 32 * s + 32)
                            ws = slice((s * 3) * 32, (s * 3 + 1) * 32)
                            nc.tensor.matmul(zt[r0, 0:GSZ], lhsT=wz, rhs=yv,
                                             start=True, stop=False,
                                             skip_group_check=True,
                                             tile_position=(tp0, 32 * s))
                            nc.tensor.matmul(zt[r0, 0:GSZ],
                                             lhsT=W2[:, ws],
                                             rhs=H[:, qc],
                                             start=False, stop=True,
                                             skip_group_check=True,
                                             tile_position=(0, 32 * s))
                            ws = slice((s * 3 + 1) * 32, (s * 3 + 2) * 32)
                            nc.tensor.matmul(rt[r0, 0:GSZ], lhsT=wr,
                                             rhs=yv, start=True, stop=False,
                                             skip_group_check=True,
                                             tile_position=(tp0, 32 * s))
                            nc.tensor.matmul(rt[r0, 0:GSZ],
                                             lhsT=W2[:, ws],
                                             rhs=H[:, qc],
                                             start=False, stop=True,
                                             skip_group_check=True,
                                             tile_position=(0, 32 * s))
                            nc.tensor.matmul(hcp[r0, 0:GSZ], lhsT=wh, rhs=yv,
                                             start=True, stop=False,
                                             skip_group_check=True,
                                             tile_position=(tp0, 32 * s))
                        nc.scalar.activation(Z[:, qc], zt[:, 0:GSZ],
                                             AF.Sigmoid, bias=BZ)
                        rq = rpool.tile([128, GSZ], bf, tag="rq")
                        nc.scalar.activation(rq[:], rt[:, 0:GSZ],
                                             AF.Sigmoid, bias=BR)
                        rhq = rpool.tile([128, GSZ], bf, tag="rhq")
                        nc.vector.tensor_tensor(out=rhq[:], in0=rq[:],
                                                in1=H[:, qc], op=ALU.mult)
                        for s in range(4):
                            r0 = slice(32 * s, 32 * s + 32)
                            ws = slice((s * 3 + 2) * 32, (s * 3 + 3) * 32)
                            nc.tensor.matmul(hcp[r0, 0:GSZ],
                                             lhsT=W2[:, ws],
                                             rhs=rhq[:, :],
                                             start=False, stop=True,
                                             skip_group_check=True,
                                             tile_position=(0, 32 * s))
                        nc.scalar.activation(HC[:, qc], hcp[:, 0:GSZ],
                                             AF.Tanh, bias=BH)
                    # h update over this quarter's nodes
                    nc.vector.tensor_tensor(out=T1[:, qcols], in0=H[:, qcols],
                                            in1=HC[:, qcols], op=ALU.subtract)
                    nc.vector.tensor_tensor(out=T2[:, qcols], in0=Z[:, qcols],
                                            in1=T1[:, qcols], op=ALU.mult)
                    nc.vector.tensor_tensor(out=H[:, qcols], in0=HC[:, qcols],
                                            in1=T2[:, qcols], op=ALU.add)
                    nc.vector.tensor_scalar(out=T1[:, qcols], in0=H[:, qcols],
                                            scalar1=float(probs[t]),
                                            scalar2=None, op0=ALU.mult)
                    nc.vector.tensor_tensor(out=ACC[:, qcols],
                                            in0=ACC[:, qcols],
                                            in1=T1[:, qcols], op=ALU.add)
                # ---- head for this quarter
                if phases >= 3:
                    nc.vector.tensor_scalar(out=T1[:, qcols],
                                            in0=ACC[:, qcols], scalar1=0.0,
                                            scalar2=None, op0=ALU.max)
                    ncols = NQ * GSZ
                    for s in range(4):
                        for c0 in range(Q * GSZ, (Q + 1) * GSZ, 512):
                            cw = min(512, (Q + 1) * GSZ - c0)
                            hp = hcpsp.tile([128, 512], f32, space="PSUM",
                                            tag="hc")
                            nc.tensor.matmul(hp[0:1, 0:cw],
                                             lhsT=HD[:, s:s + 1],
                                             rhs=T1[:, c0:c0 + cw],
                                             start=True, stop=True,
                                             skip_group_check=True,
                                             tile_position=(0, 0))
                            ot = opool.tile([1, 512], bf, tag="ot")
                            nc.scalar.activation(ot[0:1, 0:cw],
                                                 hp[0:1, 0:cw],
                                                 AF.Sigmoid,
                                                 bias=float(head_bd))
                            nc.sync.dma_start(
                                out.ap()[0:1,
                                         s * ncols + c0:s * ncols + c0 + cw],
                                ot[0:1, 0:cw])

            # quarter boundaries: emit phase-2 quarter Q once all windows
            # covering nodes < (Q+1)*2*4*GSZ are flushed
            qbound = {}
            for Q in range(NQ):
                hi_node = (Q + 1) * 4 * GSZ
                b_needed = min(cfg.NB - 1,
                               math.ceil(hi_node / (cfg.W * cfg.B)) - 1)
                if Q == NQ - 1:
                    b_needed = cfg.NB - 1
                qbound.setdefault(b_needed, []).append(Q)

            for b in range(cfg.NB):
                gs = list(range(b * cfg.B, min((b + 1) * cfg.B, cfg.NW)))
                # one window per PSUM tile (start=True zeroes the whole
                # 2KB zero region, so windows must not share a bank)
                wtiles = {}
                for g in gs:
                    pt = wpsp.tile([128, 192], f32, tag="wps")
                    wtiles[g] = (pt, 0)
                started = set()
                ends = {g: 1 + sum(int(nchunks[g * cfg.KCH + kk])
                                   for kk in range(cfg.KCH)) for g in gs}
                done = {g: 0 for g in gs}
                # self-loop contribution: contiguous local rows, no gather
                for g in gs:
                    xw = xlpool.tile([128, cfg.FPAD], bf, tag="xw")
                    nc.sync.dma_start(
                        xw[:], xloc.ap()[g * cfg.W:g * cfg.W + 128, :])
                    pt, po = wtiles[g]
                    nc.tensor.matmul(out=pt[:, po:po + 192], lhsT=IDN,
                                     rhs=xw[:, 0:192], start=True, stop=False)
                    started.add(g)
                    done[g] = 1

                # batched one-hot S tiles for this batch's chunk columns
                c_lo, c_hi = batch_cols[b]
                sb_tiles = {}
                for c0 in range(c_lo, c_hi, 16):
                    cnt = min(16, c_hi - c0)
                    sb = spool.tile([128, 16, 128], bf, tag="sb")
                    dv = DSTL[:, c0:c0 + cnt]
                    dvb = AP(dv.tensor, dv.offset,
                             [list(dv.ap[0]), list(dv.ap[1]), [0, 128]])
                    nc.vector.tensor_tensor(out=sb[:, 0:cnt, :],
                                            in0=IOTA[:, 0:cnt, :],
                                            in1=dvb, op=ALU.is_equal)
                    sb_tiles[c0] = sb

                for (bb, k, nch, wins, coff, ioff) in [e for e in bk_chunks
                                                       if e[0] == b]:
                    nidx = nch * 128
                    it = ipool.tile([128, nidx // 16], i16, tag="idx")
                    nc.sync.dma_start(it[:], idxw.ap()[:, ioff // 16:
                                                       (ioff + nidx) // 16])
                    gt = gpool.tile([128, maxnc, cfg.FPAD], bf, tag="g")
                    nc.gpsimd.dma_gather(
                        out_ap=gt[:, 0:nch, :],
                        in_ap=xap[k * cfg.CHROWS:(k + 1) * cfg.CHROWS, :],
                        idxs_ap=it[:],
                        num_idxs=nidx,
                        num_idxs_reg=nidx,
                        elem_size=cfg.FPAD,
                        single_packet=False,
                    )
                    ci = 0
                    for (g, nchw) in wins:
                        pt, po = wtiles[g]
                        for _ in range(nchw):
                            cc = coff + ci
                            sb = sb_tiles[c_lo + ((cc - c_lo) // 16) * 16]
                            S = sb[:, (cc - c_lo) % 16, :]
                            done[g] += 1
                            nc.tensor.matmul(
                                out=pt[:, po:po + 192],
                                lhsT=S,
                                rhs=gt[:, ci, 0:192],
                                start=(g not in started),
                                stop=(done[g] == ends[g]),
                            )
                            started.add(g)
                            ci += 1

                # flush: scale by dinv, transpose into A_t/B_t
                for g in gs:
                    pt, po = wtiles[g]
                    ys = ypool.tile([128, 192], bf, tag="y")
                    nc.scalar.activation(ys[:], pt[:, po:po + 192], AF.Copy,
                                         scale=DINV[:, g:g + 1])
                    tt = tpsp.tile([128, 256], bf, space="PSUM", tag="tt")
                    nc.tensor.transpose(tt[:, 0:128], ys[:, 0:128], IDN)
                    nc.tensor.transpose(tt[0:64, 128:256], ys[:, 128:192],
                                        IDN)
                    c0 = g * cfg.W
                    nc.vector.tensor_copy(A_t[:, c0:c0 + 128], tt[:, 0:128])
                    nc.vector.tensor_copy(B_t[:, c0:c0 + 128],
                                          tt[0:64, 128:256])

                for Q in qbound.get(b, []):
                    emit_phase2_quarter(Q)

            if phases == 1:
                nc.sync.dma_start(dbg.ap(), A_t)
                nc.sync.dma_start(out.ap(), A_t[0:1, :])
            if phases == 2:
                nc.sync.dma_start(dbg.ap()[:, 0:NQ * GSZ], ACC)
                nc.sync.dma_start(out.ap(), A_t[0:1, :])
            p1.close()

    nc.compile()
    return nc


# ---------------------------------------------------------------- host data

def node_col_of(cfg):
    """Map node id -> output column (block layout: node block
    i = 8*(q//2) + (q%2) + 2*s  <->  out col s*NQ*GSZ + q*GSZ + off)."""
    n = np.arange(cfg.NODE_PAD)
    blk = n // cfg.GSZ
    off = n % cfg.GSZ
    q = blk // 4
    s = blk % 4
    return s * (cfg.NQ * cfg.GSZ) + q * cfg.GSZ + off


def make_inputs(cfg, x, edge_index, attention,
                conv_wz, conv_bz, conv_wr, conv_br, conv_wh, conv_bh,
                lin_wz, lin_bz, lin_wr, lin_br, lin_wh, lin_bh,
                head_w, head_b):
    """Plan + build all per-core input arrays. Returns (shared, in_maps,
    probs, head_bd)."""
    n = cfg.N
    src = np.asarray(edge_index[0], dtype=np.int64)
    dst = np.asarray(edge_index[1], dtype=np.int64)
    deg = np.bincount(dst, minlength=n).astype(np.float64) + 1.0
    dinv = (1.0 / np.sqrt(deg)).astype(np.float32)

    shared, idx16_all, dstl_all = plan(cfg, edge_index)

    # prescaled, t-major, padded features (fp8 e4m3)
    xt = np.asarray(x, dtype=np.float32)
    xtm = np.transpose(xt, (0, 2, 1)).reshape(n, F_IN * T)  # col t*16+f
    xtm = xtm * dinv[:, None]
    xp = np.zeros((n, cfg.FPAD), dtype=BF16)
    xp[:, :cfg.FP] = xtm.astype(BF16)
    xpad = np.zeros((n + 128, cfg.FPAD), dtype=BF16)
    xpad[:n] = xp

    # folded GRU weights
    W1g = [np.asarray(conv_wz) @ np.asarray(lin_wz)[:HID],
           np.asarray(conv_wr) @ np.asarray(lin_wr)[:HID],
           np.asarray(conv_wh) @ np.asarray(lin_wh)[:HID]]   # [16,32] each
    W2g = [np.asarray(lin_wz)[HID:], np.asarray(lin_wr)[HID:],
           np.asarray(lin_wh)[HID:]]                          # [32,32] each
    bg = [np.asarray(conv_bz) @ np.asarray(lin_wz)[:HID] + np.asarray(lin_bz),
          np.asarray(conv_br) @ np.asarray(lin_wr)[:HID] + np.asarray(lin_br),
          np.asarray(conv_bh) @ np.asarray(lin_wh)[:HID] + np.asarray(lin_bh)]

    # W1big: per (t, gate) a [128, 32] column block; only the 16 rows of
    # timestep t's feature slice (within A_t / B_t) are nonzero.
    w1big = np.zeros((128, 12 * 3 * 32), dtype=np.float32)
    for t in range(T):
        tt = t if t < 8 else t - 8
        r = 32 * (tt // 2) + 16 * (tt % 2)
        for gate in range(3):
            w1big[r:r + 16, (t * 3 + gate) * 32:(t * 3 + gate + 1) * 32] = \
                W1g[gate]
    # W2big: per (s, gate) a [128, 32] block nonzero only at rows 32s..
    w2big = np.zeros((128, 4 * 3 * 32), dtype=np.float32)
    for s in range(4):
        for gate in range(3):
            w2big[32 * s:32 * s + 32,
                  (s * 3 + gate) * 32:(s * 3 + gate + 1) * 32] = W2g[gate]
    bz = np.tile(bg[0], 4)[:, None].astype(np.float32)
    br = np.tile(bg[1], 4)[:, None].astype(np.float32)
    bh = np.tile(bg[2], 4)[:, None].astype(np.float32)

    hw = np.asarray(head_w, dtype=np.float32)
    hb = np.asarray(head_b, dtype=np.float32)
    hd = np.zeros((128, 4), dtype=np.float32)
    for s in range(4):
        hd[32 * s:32 * s + 32, s] = hw[:, 1] - hw[:, 0]
    head_bd = float(hb[1] - hb[0])

    a = np.asarray(attention, dtype=np.float64)
    e = np.exp(a - a.max())
    probs = (e / e.sum()).astype(np.float32)

    iota = np.broadcast_to(np.tile(np.arange(128, dtype=np.float32), 16),
                           (128, 16 * 128))
    ident = np.eye(128, dtype=np.float32)

    # per-core dinv layout [128, NW]
    in_maps = []
    for c in range(NCORES):
        dv = np.zeros((128, cfg.NW), dtype=np.float32)
        for g in range(cfg.NW):
            lo = g * cfg.W
            hi = min(lo + cfg.W, cfg.NLOC)
            if hi > lo:
                dv[0:hi - lo, g] = dinv[c * cfg.NLOC + lo:c * cfg.NLOC + hi]
        idxc = idx16_all[c]
        idxwr = np.tile(np.ascontiguousarray(idxc.reshape(-1, 16).T), (8, 1))
        dstlc = np.ascontiguousarray(
            dstl_all[c].reshape(-1, 128).T).astype(BF16)
        in_maps.append({
            "xp": xp,
            "xloc": np.ascontiguousarray(
                xpad[c * cfg.NLOC:c * cfg.NLOC + cfg.NLOC + 128]),
            "idxw": np.ascontiguousarray(idxwr),
            "dstl": dstlc,
            "dinv": dv,
            "w1big": w1big.astype(BF16),
            "w2big": w2big.astype(BF16),
            "bz": bz, "br": br, "bh": bh,
            "hd": hd.astype(BF16),
            "iota": np.ascontiguousarray(iota).astype(BF16),
            "ident": ident.astype(BF16),
        })
    return shared, in_maps, probs, head_bd


_CACHE = {}


def _get_program(cfg, shared, probs, head_bd):
    key = ("v4", cfg.N, shared["tot"],
           tuple(np.asarray(probs).tolist()), head_bd)
    if key not in _CACHE:
        _CACHE[key] = build_program(cfg, shared, probs, head_bd)
    return _CACHE[key]


def kernel(x, edge_index, attention,
           conv_wz, conv_bz, conv_wr, conv_br, conv_wh, conv_bh,
           lin_wz, lin_bz, lin_wr, lin_br, lin_wh, lin_bh,
           head_w, head_b, _trace=False, _cfg=None):
    from concourse import bass_utils

    cfg = _cfg or FULL
    shared, in_maps, probs, head_bd = make_inputs(
        cfg, x, edge_index, attention,
        conv_wz, conv_bz, conv_wr, conv_br, conv_wh, conv_bh,
        lin_wz, lin_bz, lin_wr, lin_br, lin_wh, lin_bh, head_w, head_b)
    nc = _get_program(cfg, shared, probs, head_bd)
    res = bass_utils.run_bass_kernel_spmd(
        nc, in_maps, core_ids=list(range(NCORES)), trace=_trace)

    colmap = node_col_of(cfg)[:cfg.NLOC]
    p1 = np.concatenate(
        [np.asarray(r["out"][0], dtype=np.float32)[colmap]
         for r in res.results])
    outp = np.empty((cfg.N, N_CLS), dtype=np.float32)
    outp[:, 1] = p1
    outp[:, 0] = 1.0 - p1
    if _trace:
        return outp, res
    return outp
